# revision 9
# baseline (speedup 1.0000x reference)
"""Trainium2 Bass kernel for nn_AttentionUnit (attention pooling).

reference math:
    q = query @ Wq.T + bq                      [B, D]
    k = keys @ Wk.T + bk                       [B, L, D]
    score[b,l] = <k[b,l], q[b]>
    attn = softmax(score, axis=1)              [B, L]
    wsum[b] = sum_l attn[b,l] * keys[b,l]      [B, D]
    return (wsum, attn)

Algebraic fold: score[b,l] = <keys[b,l,:], qk[b,:]> + c[b] with
qk = (query @ Wq.T + bq) @ Wk and c[b] = <q[b], bk>.  c[b] is constant over
l so it drops out of the softmax -> bk is irrelevant to the output, and the
huge keys-projection matmul disappears.  The kernel is memory-bound on
reading keys (~105 MB fp32 per core).

Both remaining contractions run on the tensor engine, which contracts along
the partition dim only -- so keys are staged in two layouts:
  - keysT fp32 [B, D, L] (host-pretransposed): score[b,l] via per-b matmuls
    lhsT = keysT_b[:, l-chunk] (i on partitions), rhs = qkT[:, b].
  - keysN fp16 [B, 100, 2*D] (l and l+100 paired per row): wsum via per-b
    matmuls lhsT = keysN chunk (l on partitions), rhs = expT fp16 column.
    fp16 keys (rel eps 2^-12) keep wsum err ~1e-4; exp weights are
    max-subtracted so they lie in (0,1], safely inside fp16 range.
softmax: scoreT [l,b] chunks -> PE transpose -> [b,l], DVE reduce_max,
exp via ACT with bias=-max and accumulated row sums, DVE reciprocal.
"""

import sys

sys.path.insert(0, "/opt/trn_rl_repo")

import numpy as np

D = 128
L = 200
LH = L // 2  # 100, the l-chunk size
B_FULL = 8192
N_CORES = 8
B_CORE = B_FULL // N_CORES  # 1024

_NC_CACHE = {}


def build_nc(b_core=B_CORE, tile_b=32, kt_fp16=False):
    """Build the single-core Bass program (same program runs SPMD on all cores)."""
    from contextlib import ExitStack

    import concourse.masks as masks
    import concourse.mybir as mybir
    import concourse.tile as tile
    from concourse.bacc import Bacc

    f32 = mybir.dt.float32
    f16 = mybir.dt.float16
    ktd = f16 if kt_fp16 else f32
    AF = mybir.ActivationFunctionType
    OP = mybir.AluOpType

    assert b_core % tile_b == 0
    n_tiles = b_core // tile_b
    assert b_core % 128 == 0

    nc = Bacc()

    keysT_d = nc.dram_tensor("keysT", [b_core, D, L], ktd, kind="ExternalInput")
    keysN_d = nc.dram_tensor("keysN", [b_core, LH, 2 * D], f16, kind="ExternalInput")
    queryT_d = nc.dram_tensor("queryT", [D, b_core], f32, kind="ExternalInput")
    wqT_d = nc.dram_tensor("wqT", [D, D], f32, kind="ExternalInput")
    wk_d = nc.dram_tensor("wk", [D, D], f32, kind="ExternalInput")
    bq_d = nc.dram_tensor("bq", [D, 1], f32, kind="ExternalInput")
    wsum_d = nc.dram_tensor("wsum", [b_core, D], f32, kind="ExternalOutput")
    attn_d = nc.dram_tensor("attn", [b_core, L], f32, kind="ExternalOutput")

    with tile.TileContext(nc) as tc, ExitStack() as ctx:
        const = ctx.enter_context(tc.tile_pool(name="const", bufs=1))

        identity = const.tile([128, 128], f32)
        masks.make_identity(nc, identity[:])
        queryT_s = const.tile([D, b_core], f32)
        nc.sync.dma_start(queryT_s[:], queryT_d[:])
        # qkT[i, b] = sum_e Wk[e, i] * (Wq q + bq)[e, b]
        qkT_all = const.tile([D, b_core], ktd)

        # SBUF pools
        kt_p = ctx.enter_context(tc.tile_pool(name="kt", bufs=2))
        kn_p = ctx.enter_context(tc.tile_pool(name="kn", bufs=3))
        sm_p = ctx.enter_context(tc.tile_pool(name="sm", bufs=3))
        out_p = ctx.enter_context(tc.tile_pool(name="outs", bufs=2))
        # PSUM pools (8 banks total: 2+2+2+2)
        ps_sc = ctx.enter_context(tc.tile_pool(name="ps_sc", bufs=2, space="PSUM"))
        ps_tr = ctx.enter_context(tc.tile_pool(name="ps_tr", bufs=2, space="PSUM"))
        ps_et = ctx.enter_context(tc.tile_pool(name="ps_et", bufs=2, space="PSUM"))
        ps_uw = ctx.enter_context(tc.tile_pool(name="ps_uw", bufs=2, space="PSUM"))

        # ---------------- pre-pass: qkT = Wk.T-contract @ (WqT.T @ queryT + bq)
        with tc.tile_pool(name="pre_sb", bufs=2) as pre_sb:
            wqT_s = pre_sb.tile([D, D], f32, tag="w")
            nc.sync.dma_start(wqT_s[:], wqT_d[:])
            wk_s = pre_sb.tile([D, D], f32, tag="w")
            nc.sync.dma_start(wk_s[:], wk_d[:])
            bq_s = pre_sb.tile([D, 1], f32, tag="b")
            nc.sync.dma_start(bq_s[:], bq_d[:])
            for t8 in range(b_core // 128):
                sl = slice(t8 * 128, (t8 + 1) * 128)
                qT_ps = ps_sc.tile([128, 128], f32, tag="sc")
                nc.tensor.matmul(qT_ps[:], wqT_s[:], queryT_s[:, sl])
                qT_sb = pre_sb.tile([128, 128], f32, tag="q")
                nc.scalar.activation(qT_sb[:], qT_ps[:], AF.Identity, bias=bq_s[:, 0:1])
                qk_ps = ps_sc.tile([128, 128], f32, tag="sc")
                nc.tensor.matmul(qk_ps[:], wk_s[:], qT_sb[:])
                nc.scalar.activation(qkT_all[:, sl], qk_ps[:], AF.Copy, bias=0.0)

        # ---------------- main pipeline over b-tiles -----------------------
        state = {}

        def emit_front(t):
            t0 = t * tile_b
            kT = kt_p.tile([D, tile_b, L], ktd, tag="kt")
            nc.sync.dma_start(
                kT[:], keysT_d[t0 : t0 + tile_b, :, :].rearrange("b i l -> i b l")
            )
            kN = kn_p.tile([LH, tile_b, 2 * D], f16, tag="kn")
            nc.sync.dma_start(
                kN[:], keysN_d[t0 : t0 + tile_b, :, :].rearrange("b l x -> l b x")
            )

            # scores, transposed: scoreT[l, j] columns per batch row j
            scA_ps = ps_sc.tile([LH, tile_b], f32, tag="sc")
            scB_ps = ps_sc.tile([LH, tile_b], f32, tag="sc")
            for j in range(tile_b):
                rhs = qkT_all[:, t0 + j : t0 + j + 1]
                nc.tensor.matmul(scA_ps[:, j : j + 1], kT[:, j, 0:LH], rhs)
                nc.tensor.matmul(scB_ps[:, j : j + 1], kT[:, j, LH:L], rhs)
            scA_sb = sm_p.tile([LH, tile_b], f32, tag="scs")
            nc.scalar.activation(scA_sb[:], scA_ps[:], AF.Copy, bias=0.0)
            scB_sb = sm_p.tile([LH, tile_b], f32, tag="scs")
            nc.scalar.activation(scB_sb[:], scB_ps[:], AF.Copy, bias=0.0)
            # transpose to [b, l] chunks
            trA = ps_tr.tile([tile_b, LH], f32, tag="tr")
            nc.tensor.transpose(trA[:], scA_sb[:], identity[0:LH, 0:LH])
            trB = ps_tr.tile([tile_b, LH], f32, tag="tr")
            nc.tensor.transpose(trB[:], scB_sb[:], identity[0:LH, 0:LH])

            # softmax pieces: per-row max, exp(s - max) with accumulated sums
            mA = sm_p.tile([tile_b, 1], f32, tag="m")
            nc.vector.tensor_reduce(mA[:], trA[:], axis=mybir.AxisListType.X, op=OP.max)
            mB = sm_p.tile([tile_b, 1], f32, tag="m")
            nc.vector.tensor_reduce(mB[:], trB[:], axis=mybir.AxisListType.X, op=OP.max)
            negm = sm_p.tile([tile_b, 1], f32, tag="m")
            nc.vector.tensor_tensor(negm[:], mA[:], mB[:], op=OP.max)
            nc.vector.tensor_scalar_mul(negm[:], negm[:], -1.0)
            exp_sb = sm_p.tile([tile_b, L], f32, tag="e")
            sA = sm_p.tile([tile_b, 1], f32, tag="s")
            nc.scalar.activation(
                exp_sb[:, 0:LH], trA[:], AF.Exp, bias=negm[:, 0:1], accum_out=sA[:, 0:1]
            )
            sB = sm_p.tile([tile_b, 1], f32, tag="s")
            nc.scalar.activation(
                exp_sb[:, LH:L], trB[:], AF.Exp, bias=negm[:, 0:1], accum_out=sB[:, 0:1]
            )
            rsum = sm_p.tile([tile_b, 1], f32, tag="r")
            nc.vector.tensor_tensor(rsum[:], sA[:], sB[:], op=OP.add)
            nc.vector.reciprocal(rsum[:], rsum[:])

            # attn output straight from [b, l] layout
            attn_sb = out_p.tile([tile_b, L], f32, tag="at")
            nc.vector.tensor_scalar_mul(attn_sb[:], exp_sb[:], rsum[:, 0:1])
            nc.scalar.dma_start(attn_d[t0 : t0 + tile_b, :], attn_sb[:])

            # transposed fp16 exp columns for the wsum matmuls
            etA_ps = ps_et.tile([LH, tile_b], f32, tag="et")
            nc.tensor.transpose(etA_ps[:], exp_sb[:, 0:LH], identity[0:tile_b, 0:tile_b])
            etB_ps = ps_et.tile([LH, tile_b], f32, tag="et")
            nc.tensor.transpose(etB_ps[:], exp_sb[:, LH:L], identity[0:tile_b, 0:tile_b])
            etA = sm_p.tile([LH, tile_b], f16, tag="et16")
            nc.scalar.activation(etA[:], etA_ps[:], AF.Copy, bias=0.0)
            etB = sm_p.tile([LH, tile_b], f16, tag="et16")
            nc.scalar.activation(etB[:], etB_ps[:], AF.Copy, bias=0.0)

            state[t] = (kN, etA, etB, rsum)

        def emit_back(t):
            t0 = t * tile_b
            kN, etA, etB, rsum = state.pop(t)
            uwsT_ps = ps_uw.tile([D, tile_b], f32, tag="uw")
            for j in range(tile_b):
                nc.tensor.matmul(
                    uwsT_ps[:, j : j + 1],
                    kN[:, j, 0:D],
                    etA[:, j : j + 1],
                    start=True,
                    stop=False,
                )
                nc.tensor.matmul(
                    uwsT_ps[:, j : j + 1],
                    kN[:, j, D : 2 * D],
                    etB[:, j : j + 1],
                    start=False,
                    stop=True,
                )
            uwsT_sb = out_p.tile([D, tile_b], f32, tag="uwc")
            nc.scalar.activation(uwsT_sb[:], uwsT_ps[:], AF.Copy, bias=0.0)
            uws_ps = ps_tr.tile([tile_b, D], f32, tag="tr")
            nc.tensor.transpose(uws_ps[:], uwsT_sb[:], identity[:])
            wsum_sb = out_p.tile([tile_b, D], f32, tag="ws")
            nc.vector.tensor_scalar_mul(wsum_sb[:], uws_ps[:], rsum[:, 0:1])
            nc.scalar.dma_start(wsum_d[t0 : t0 + tile_b, :], wsum_sb[:])

        for t in range(n_tiles):
            emit_front(t)
            if t > 0:
                emit_back(t - 1)
        emit_back(n_tiles - 1)

    nc.finalize()
    return nc


def get_nc(**kw):
    key = tuple(sorted(kw.items()))
    if key not in _NC_CACHE:
        _NC_CACHE[key] = build_nc(**kw)
    return _NC_CACHE[key]


def _prep_in_maps(query, keys, Wq, bq, Wk, b_core=B_CORE, kt_fp16=False):
    query = np.asarray(query, dtype=np.float32)
    keys = np.asarray(keys, dtype=np.float32)
    B = query.shape[0]
    ktd = np.float16 if kt_fp16 else np.float32
    keysT = np.ascontiguousarray(keys.transpose(0, 2, 1)).astype(ktd)  # [B, D, L]
    # pair rows l and l+100 so each native fp16 row is 512B contiguous
    keysN = np.ascontiguousarray(
        keys.reshape(B, 2, LH, D).transpose(0, 2, 1, 3).reshape(B, LH, 2 * D)
    ).astype(np.float16)
    wqT = np.ascontiguousarray(np.asarray(Wq, dtype=np.float32).T)
    wk = np.ascontiguousarray(np.asarray(Wk, dtype=np.float32))
    bqc = np.ascontiguousarray(np.asarray(bq, dtype=np.float32).reshape(D, 1))
    queryT = np.ascontiguousarray(query.T)  # [D, B]
    in_maps = []
    for c in range(B // b_core):
        s = slice(c * b_core, (c + 1) * b_core)
        in_maps.append(
            {
                "keysT": keysT[s],
                "keysN": keysN[s],
                "queryT": np.ascontiguousarray(queryT[:, s]),
                "wqT": wqT,
                "wk": wk,
                "bq": bqc,
            }
        )
    return in_maps


def kernel(query, keys, Wq, bq, Wk, bk):
    from concourse.bass_utils import run_bass_kernel_spmd

    nc = get_nc()
    in_maps = _prep_in_maps(query, keys, Wq, bq, Wk)
    res = run_bass_kernel_spmd(nc, in_maps, list(range(N_CORES))).results
    wsum = np.concatenate([r["wsum"] for r in res], axis=0)
    attn = np.concatenate([r["attn"] for r in res], axis=0)
    return (wsum, attn)


# ----------------------------------------------------------------------------
def _np_reference(query, keys, Wq, bq, Wk, bk):
    q = query @ Wq.T + bq
    k = keys @ Wk.T + bk
    score = np.einsum("bld,bd->bl", k, q)
    m = score.max(axis=1, keepdims=True)
    e = np.exp(score - m)
    attn = e / e.sum(axis=1, keepdims=True)
    ws = np.einsum("bl,bld->bd", attn, keys)
    return ws, attn


def _make_inputs(b, seed=0):
    rng = np.random.default_rng(seed)
    s = 1.0 / np.sqrt(D)
    return dict(
        query=rng.standard_normal((b, D), dtype=np.float32),
        keys=rng.standard_normal((b, L, D), dtype=np.float32),
        Wq=rng.standard_normal((D, D), dtype=np.float32) * s,
        bq=rng.standard_normal((D,), dtype=np.float32) * s,
        Wk=rng.standard_normal((D, D), dtype=np.float32) * s,
        bk=rng.standard_normal((D,), dtype=np.float32) * s,
    )


def _selftest_sim(b_core=128, tile_b=32, kt_fp16=False):
    from concourse.bass_interp import CoreSim

    ins = _make_inputs(b_core)
    nc = build_nc(b_core=b_core, tile_b=tile_b, kt_fp16=kt_fp16)
    in_map = _prep_in_maps(
        ins["query"], ins["keys"], ins["Wq"], ins["bq"], ins["Wk"],
        b_core=b_core, kt_fp16=kt_fp16,
    )[0]
    sim = CoreSim(nc)
    for name, arr in in_map.items():
        sim.tensor(name)[:] = arr
    sim.simulate()
    wsum = np.array(sim.tensor("wsum"))
    attn = np.array(sim.tensor("attn"))
    ws_ref, attn_ref = _np_reference(
        ins["query"], ins["keys"], ins["Wq"], ins["bq"], ins["Wk"], ins["bk"]
    )
    for name, got, ref, tol in (
        ("wsum", wsum, ws_ref, 2e-3),
        ("attn", attn, attn_ref, 2e-2 if kt_fp16 else 2e-4),
    ):
        err = np.abs(got - ref).max() / (np.abs(ref).max() + 1e-30)
        print(f"{name}: rel abs-max err = {err:.3e}")
        assert err < tol, f"{name} mismatch"
    print("SIM SELFTEST PASSED")


if __name__ == "__main__":
    _selftest_sim()
